# revision 32
# baseline (speedup 1.0000x reference)
"""Trainium2 Bass kernel for nn_Attention_10711648436709.

Math (faithful to reference):
    h = einsum('bhik,bhjk->bhij', Q, K) / sqrt(H)     # scale = sqrt(16) = 4
    w = softmax(h, axis=0)                            # over the BATCH axis (B=4)
    out = einsum('bhij,bhjv->bhiv', w, V)
    (mask is a no-op in the reference)

Sharding: head-parallel across 8 cores (16 heads -> 2 heads/core).
Softmax over batch stays core-local => communication-free.

Per-core layout trick: compute transposed scores S^T[j, i] so that
 - QK:  lhsT = K^T[d, j-block]  rhs = Q^T[d, i-chunk]   (host pre-transposes Q,K)
 - PV:  lhsT = V[j-block, v]    rhs = W[j, i-chunk]     (V in natural layout)
 - output accumulates as out^T[v, i] in PSUM; host transposes back.
Softmax over batch is elementwise across the 4 per-batch score planes that
live side-by-side in one [128, 4*512] PSUM mega-tile (one exp covers all 4).
"""

import sys
import os

for p in ("/opt/trn_rl_repo",):
    if p not in sys.path:
        sys.path.insert(0, p)

import numpy as np
import ml_dtypes

B, H, S, D = 4, 16, 2048, 64
NCORES = 8
HL = H // NCORES          # 2 heads per core
NB = S // 128             # 16 j-blocks
NI = S // 512             # 4 i-chunks

TRACE = False
LAST_EXEC_NS = None
LAST_RESULTS = None

_NC = None


def _build_nc():
    import concourse.bass as bass
    import concourse.mybir as mybir
    import concourse.tile as tile

    DT = mybir.dt
    AF = mybir.ActivationFunctionType
    ALU = mybir.AluOpType

    nc = bass.Bass()
    # Batch-0-pivot softmax: g_b = h_b - h_0 (b=1..3) computed by ONE
    # full-K=128 matmul each: lhsT = [K_b^T ; K_0^T], rhs = [Q_b^T ; -Q_0^T]
    # (host packs/negates). Then w_b = e^{g_b/4}/(1 + sum e^{g_b'/4}) and
    # w_0 = 1/(1 + sum ...) = r, so batch 0 needs no exp and no multiply.
    qt = nc.declare_dram_parameter("qt", [3, HL, 128, S], DT.bfloat16, isOutput=False)
    kt = nc.declare_dram_parameter("kt", [3, HL, 128, S], DT.bfloat16, isOutput=False)
    vv = nc.declare_dram_parameter("v", [B, HL, S, D], DT.bfloat16, isOutput=False)
    # negated V: on DVE-reciprocal iterations the Newton chain produces -r
    # (one op shorter); pairing those weights with -V cancels the sign in
    # the PV accumulation.
    vn = nc.declare_dram_parameter("vn", [B, HL, S, D], DT.bfloat16, isOutput=False)
    out = nc.declare_dram_parameter("out", [B, HL, D, S], DT.float32, isOutput=True)

    with tile.TileContext(nc) as tc:
        with (
            tc.tile_pool(name="inputs", bufs=1) as ipool,
            tc.tile_pool(name="work", bufs=8) as wpool,
            tc.tile_pool(name="outsb", bufs=4) as opool,
            tc.tile_pool(name="qkps", bufs=1, space="PSUM") as qkpool,
            tc.tile_pool(name="ops", bufs=2, space="PSUM") as opsum,
        ):
            QT = ipool.tile([128, 3 * HL * S], DT.bfloat16, tag="qt")
            KT = ipool.tile([128, 3 * HL * S], DT.bfloat16, tag="kt")
            VA = ipool.tile([128, B * HL * NB * D], DT.bfloat16, tag="va")
            VN = ipool.tile([128, B * HL * NB * D], DT.bfloat16, tag="vn")
            for bb in range(3):
                for hl in range(HL):
                    off = (bb * HL + hl) * S
                    nc.sync.dma_start(out=QT[:, off : off + S], in_=qt[bb, hl])
                    nc.sync.dma_start(out=KT[:, off : off + S], in_=kt[bb, hl])
            for b in range(B):
                for hl in range(HL):
                    voff = (b * HL + hl) * NB * D
                    nc.sync.dma_start(
                        out=VA[:, voff : voff + NB * D].rearrange(
                            "p (n d) -> p n d", d=D
                        ),
                        in_=vv[b, hl].rearrange("(n p) d -> p n d", p=128),
                    )
                    nc.sync.dma_start(
                        out=VN[:, voff : voff + NB * D].rearrange(
                            "p (n d) -> p n d", d=D
                        ),
                        in_=vn[b, hl].rearrange("(n p) d -> p n d", p=128),
                    )

            for hl in range(HL):
                for ic in range(NI):
                    po = [
                        opsum.tile(
                            [128, 512], DT.float32, tag=f"po{p}", name=f"po{p}"
                        )
                        for p in range(2)
                    ]
                    for jb in range(NB):
                        # [128, 3*512] = 3 banks holding g_1|g_2|g_3;
                        # bufs=2 double-buffers the QK->exp handoff
                        qk = qkpool.tile([128, 1536], DT.float32, tag="qk")
                        for bb in range(3):
                            off = (bb * HL + hl) * S
                            nc.tensor.matmul(
                                qk[:, bb * 512 : (bb + 1) * 512],
                                lhsT=KT[:, off + jb * 128 : off + jb * 128 + 128],
                                rhs=QT[:, off + ic * 512 : off + ic * 512 + 512],
                                start=True,
                                stop=True,
                            )
                        E = wpool.tile([128, 1536], DT.bfloat16, tag="E")
                        nc.scalar.activation(E, qk, AF.Exp, scale=0.25)
                        T1 = wpool.tile([128, 512], DT.bfloat16, tag="T1")
                        nc.vector.tensor_add(T1, E[:, 0:512], E[:, 512:1024])
                        den = wpool.tile([128, 512], DT.bfloat16, tag="den")
                        # den = (E3 + 1.0) + T1 in one fused op
                        nc.vector.scalar_tensor_tensor(
                            out=den,
                            in0=E[:, 1024:1536],
                            scalar=1.0,
                            in1=T1,
                            op0=ALU.add,
                            op1=ALU.add,
                        )
                        rb = wpool.tile([128, 512], DT.bfloat16, tag="rb")
                        act_recip = jb % 2 == 0
                        if act_recip:
                            # reciprocal on ACT: r = exp(-ln(den)); Ln and Exp
                            # share the natural_log_exp table set
                            lnt = wpool.tile([128, 512], DT.float32, tag="lnt")
                            nc.scalar.activation(lnt, den, AF.Ln)
                            nc.scalar.activation(rb, lnt, AF.Exp, scale=-1.0)
                        else:
                            # reciprocal on DVE: bf16 exponent-flip seed + one
                            # Newton iteration, fused to produce -r (the PV
                            # matmuls below compensate with lhsT = -V)
                            ti = wpool.tile([128, 512], DT.bfloat16, tag="ti")
                            nc.vector.tensor_scalar(
                                out=ti.bitcast(DT.int16),
                                in0=den.bitcast(DT.int16),
                                scalar1=-1,
                                scalar2=None,
                                op0=ALU.bitwise_xor,
                            )
                            x0 = wpool.tile([128, 512], DT.bfloat16, tag="x0")
                            nc.vector.tensor_scalar(
                                out=x0.bitcast(DT.int16),
                                in0=ti.bitcast(DT.int16),
                                scalar1=0x7EF4,
                                scalar2=None,
                                op0=ALU.add,
                            )
                            dx = wpool.tile([128, 512], DT.bfloat16, tag="dx")
                            nc.vector.tensor_mul(dx, den, x0)
                            # rb = (dx - 2) * x0 = -x0*(2 - dx) = -r
                            nc.vector.scalar_tensor_tensor(
                                out=rb,
                                in0=dx,
                                scalar=2.0,
                                in1=x0,
                                op0=ALU.subtract,
                                op1=ALU.mult,
                            )
                        W = wpool.tile([128, 1536], DT.bfloat16, tag="W")
                        nc.vector.tensor_mul(
                            W.rearrange("q (b n) -> q b n", b=3),
                            E.rearrange("q (b n) -> q b n", b=3),
                            rb.unsqueeze(1).broadcast_to([128, 3, 512]),
                        )
                        # PV rhs per batch: b0 -> rb (w_0 = r), b1..b3 -> W
                        rhss = [rb, W[:, 0:512], W[:, 512:1024], W[:, 1024:1536]]
                        VV = VA if act_recip else VN
                        for p in range(2):
                            for half in range(2):
                                b = 2 * p + half
                                voff = (b * HL + hl) * NB * D + jb * D
                                nc.tensor.matmul(
                                    po[p][64 * half : 64 * (half + 1), :],
                                    lhsT=VV[:, voff : voff + D],
                                    rhs=rhss[b],
                                    start=(jb == 0),
                                    stop=(jb == NB - 1),
                                    tile_position=(0, 64 * half),
                                )
                    for p in range(2):
                        for half in range(2):
                            b = 2 * p + half
                            osb = opool.tile([D, 512], DT.float32, tag="osb")
                            # ACT, not DVE: DVE is the bottleneck engine and
                            # ScalarE reads PSUM faster anyway
                            nc.scalar.copy(
                                osb, po[p][64 * half : 64 * (half + 1), :]
                            )
                            nc.sync.dma_start(
                                out=out[b, hl, :, ic * 512 : (ic + 1) * 512],
                                in_=osb,
                            )
    return nc


def _patch_bir_waits(bir_json: bytes) -> bytes:
    """This walrus build only accepts 1 sync wait per instruction (2 for
    DMACopy); Tile emits more. Legalize:
      1. merge duplicate-semaphore waits (keep max threshold),
      2. drop waits that are transitively implied (vector-clock replay over
         the straight-line program: in-order completion per engine, FIFO per
         DMA queue, and the knowledge a producer had when it bumped a sem),
      3. split any residual multi-wait onto injected EventSemaphore
         instructions on the same engine right before the instruction.
    Only monotonic sem-inc/sem-ge-imm semaphores participate in (2); barrier
    sems (dec/eq) are left untouched."""
    import json
    from collections import defaultdict

    bir = json.loads(bir_json)

    for fn in bir["functions"]:
        insts = []
        for bb in fn["blocks"]:
            for inst in bb.get("instructions", []):
                insts.append(inst)

        # classify sems: monotonic = all updates are positive sem-inc and
        # all waits are sem-ge-imm
        bad_sems = set()
        for inst in insts:
            si = inst.get("sync_info") or {}
            for u in si.get("on_update") or []:
                if u.get("update_mode") != "sem-inc" or u.get("update_value", 0) <= 0:
                    bad_sems.add(u["id"])
            for w in si.get("on_wait") or []:
                if w.get("wait_mode") != "sem-ge-imm":
                    bad_sems.add(w["id"])

        # proc of an instruction: its engine stream, except DMACopy whose
        # completion (and sem update) is FIFO per DMA queue, keyed by the
        # sem it updates.
        def proc_of(inst):
            if inst.get("opcode") == "DMACopy":
                si = inst.get("sync_info") or {}
                ups = si.get("on_update") or []
                if ups:
                    return ("dma", ups[0]["id"])
            return ("eng", inst.get("engine"))

        sem_val = defaultdict(int)          # current cumulative value per sem
        producers = defaultdict(list)       # sem -> [(value_after, CK dict)]
        know = defaultdict(dict)            # proc -> {sem: guaranteed min}

        def join(dst, src):
            for s, v in src.items():
                if dst.get(s, 0) < v:
                    dst[s] = v

        out_blocks = {id(bb): [] for bb in fn["blocks"]}
        inj = 0
        for bb in fn["blocks"]:
            new_list = []
            for inst in bb.get("instructions", []):
                p = proc_of(inst)
                eng_p = ("eng", inst.get("engine"))
                # waits on a DMACopy are enforced by the DGE queue (FIFO per
                # queue), not the issuing engine — track knowledge per queue
                kp = p if p[0] == "dma" else eng_p
                si = inst.get("sync_info") or {}
                waits = si.get("on_wait") or []
                # merge duplicate sems
                merged = {}
                for w in waits:
                    k = w["id"]
                    if k not in merged or w.get("wait_value", 0) > merged[k].get(
                        "wait_value", 0
                    ):
                        merged[k] = w
                waits = list(merged.values())
                kept = []
                for w in waits:
                    s, v = w["id"], w.get("wait_value", 0)
                    if s in bad_sems:
                        kept.append(w)
                        continue
                    if know[kp].get(s, 0) >= v:
                        continue  # redundant
                    kept.append(w)
                    know[kp][s] = max(know[kp].get(s, 0), v)
                    # transitive knowledge from the producer that reached v
                    for val_after, ck in producers[s]:
                        if val_after >= v:
                            join(know[kp], ck)
                            break
                # split if too many waits remain
                budget = 1
                while len(kept) > budget:
                    w = kept.pop(0)
                    inj += 1
                    new_list.append(
                        {
                            "debug": inst.get("debug", 0),
                            "engine": inst.get("engine"),
                            "ins": [],
                            "name": f"WS-{inj}-{inst.get('name')}",
                            "opcode": "EventSemaphore",
                            "outs": [],
                            "sync_info": {"on_update": [], "on_wait": [w]},
                        }
                    )
                si["on_wait"] = kept
                inst["sync_info"] = si
                new_list.append(inst)
                # apply this instruction's updates for downstream knowledge
                ups = si.get("on_update") or []
                ck = None
                for u in ups:
                    s = u["id"]
                    if s in bad_sems:
                        continue
                    sem_val[s] += u.get("update_value", 0)
                    if ck is None:
                        # completion knowledge: what this proc knew here
                        # (for DMA: queue knowledge + engine state at issue)
                        ck = dict(know[kp])
                        if p[0] == "dma":
                            join(ck, know[eng_p])
                    ck[s] = sem_val[s]
                    producers[s].append((sem_val[s], ck))
                # a proc knows its own sems' values after completion
                if p[0] == "eng":
                    for u in ups:
                        if u["id"] not in bad_sems:
                            know[eng_p][u["id"]] = sem_val[u["id"]]
            out_blocks[id(bb)] = new_list
        for bb in fn["blocks"]:
            bb["instructions"] = out_blocks[id(bb)]
    return json.dumps(bir).encode()


_PATCHED = False


def _install_bir_patch():
    global _PATCHED
    if _PATCHED:
        return
    import concourse.bass2jax as bass2jax
    from concourse import bass_utils as _bu

    orig = _bu.compile_bir_kernel

    def patched(bir_json, tmpdir, neff_name="file.neff"):
        return orig(_patch_bir_waits(bir_json), tmpdir, neff_name)

    bass2jax.compile_bir_kernel = patched
    # keep profile artifacts local — no bucket in this environment
    _bu.upload_artifacts = lambda tmpdir: str(tmpdir)
    _PATCHED = True


def _install_ntff_shim():
    """run_bass_kernel_spmd(trace=True) under axon needs
    antenv.axon_hooks.get_axon_ntff_profile_hook; the module isn't staged in
    this image, but libaxon_pjrt.so exposes the profile C ABI — recreate the
    shim (same recipe as trn_agent_boot)."""
    import sys as _sys

    if "antenv.axon_hooks" in _sys.modules:
        return
    import contextlib
    import ctypes
    import types

    import antenv  # noqa: F401

    so_path = "/opt/axon/libaxon_pjrt.so"
    hook = None
    try:
        lib = ctypes.CDLL(so_path)
        if hasattr(lib, "axon_start_nrt_profile"):
            lib.axon_start_nrt_profile.argtypes = [
                ctypes.POINTER(ctypes.c_int64),
                ctypes.c_size_t,
            ]
            lib.axon_start_nrt_profile.restype = ctypes.c_int64
            lib.axon_stop_nrt_profile.argtypes = [ctypes.c_char_p]
            lib.axon_stop_nrt_profile.restype = ctypes.c_int64

            @contextlib.contextmanager
            def hook(output_dir, device_ids):
                import jax

                jax.devices()
                if device_ids:
                    ids = (ctypes.c_int64 * len(device_ids))(*device_ids)
                    rc = lib.axon_start_nrt_profile(ids, len(device_ids))
                else:
                    rc = lib.axon_start_nrt_profile(None, 0)
                if rc != 0:
                    raise RuntimeError(f"axon_start_nrt_profile rc={rc}")
                try:
                    yield
                finally:
                    n = lib.axon_stop_nrt_profile(str(output_dir).encode())
                    print(
                        f"ntff profile: {n} file(s) -> {output_dir}",
                        file=_sys.stderr,
                    )
    except OSError:
        pass

    mod = types.ModuleType("antenv.axon_hooks")
    mod.get_axon_ntff_profile_hook = lambda: hook
    mod.set_axon_ntff_profile_hook = lambda h: None
    _sys.modules["antenv.axon_hooks"] = mod
    import antenv as _ae

    _ae.axon_hooks = mod


def kernel(query, key, value, mask=None):
    global _NC, LAST_EXEC_NS, LAST_RESULTS
    from concourse.bass_utils import run_bass_kernel_spmd

    _install_bir_patch()
    if TRACE:
        _install_ntff_shim()

    query = np.asarray(query, dtype=np.float32)
    key = np.asarray(key, dtype=np.float32)
    value = np.asarray(value, dtype=np.float32)

    if _NC is None:
        _NC = _build_nc()
    nc = _NC

    bf16 = ml_dtypes.bfloat16

    def pack_pivot(x, negate_base):
        # [B, HL, S, D] -> [B, HL, D, S]; stack [x_b^T ; (+-)x_0^T] on the
        # partition axis for b = 1..3 -> [3, HL, 128, S]
        xt = x.transpose(0, 1, 3, 2)  # [B, HL, D, S]
        base = -xt[0] if negate_base else xt[0]  # [HL, D, S]
        stk = np.stack(
            [np.concatenate([xt[b], base], axis=1) for b in (1, 2, 3)], axis=0
        )
        return np.ascontiguousarray(stk).astype(bf16)

    in_maps = []
    for c in range(NCORES):
        hs = slice(HL * c, HL * (c + 1))
        qt = pack_pivot(query[:, hs], negate_base=True)
        kt = pack_pivot(key[:, hs], negate_base=False)
        vc = np.ascontiguousarray(value[:, hs]).astype(bf16)
        vnc = np.ascontiguousarray(-value[:, hs]).astype(bf16)
        in_maps.append({"qt": qt, "kt": kt, "v": vc, "vn": vnc})

    res = run_bass_kernel_spmd(
        nc, in_maps, core_ids=list(range(NCORES)), trace=TRACE
    )
    LAST_RESULTS = res
    LAST_EXEC_NS = getattr(res, "exec_time_ns", None)

    full = np.empty((B, H, S, D), dtype=np.float32)
    for c in range(NCORES):
        o = np.asarray(res.results[c]["out"])  # [B, HL, D, S]
        full[:, HL * c : HL * (c + 1)] = o.transpose(0, 1, 3, 2)
    return full


# revision 35
# speedup vs baseline: 1.0338x; 1.0338x over previous
"""Trainium2 Bass kernel for nn_Attention_10711648436709.

Math (faithful to reference):
    h = einsum('bhik,bhjk->bhij', Q, K) / sqrt(H)     # scale = sqrt(16) = 4
    w = softmax(h, axis=0)                            # over the BATCH axis (B=4)
    out = einsum('bhij,bhjv->bhiv', w, V)
    (mask is a no-op in the reference)

Sharding: head-parallel across 8 cores (16 heads -> 2 heads/core).
Softmax over batch stays core-local => communication-free.

Per-core layout trick: compute transposed scores S^T[j, i] so that
 - QK:  lhsT = K^T[d, j-block]  rhs = Q^T[d, i-chunk]   (host pre-transposes Q,K)
 - PV:  lhsT = V[j-block, v]    rhs = W[j, i-chunk]     (V in natural layout)
 - output accumulates as out^T[v, i] in PSUM; host transposes back.
Softmax over batch is elementwise across the 4 per-batch score planes that
live side-by-side in one [128, 4*512] PSUM mega-tile (one exp covers all 4).
"""

import sys
import os

for p in ("/opt/trn_rl_repo",):
    if p not in sys.path:
        sys.path.insert(0, p)

import numpy as np
import ml_dtypes

B, H, S, D = 4, 16, 2048, 64
NCORES = 8
HL = H // NCORES          # 2 heads per core
NB = S // 128             # 16 j-blocks
NI = S // 512             # 4 i-chunks

TRACE = False
LAST_EXEC_NS = None
LAST_RESULTS = None

_NC = None


def _build_nc():
    import concourse.bass as bass
    import concourse.mybir as mybir
    import concourse.tile as tile

    DT = mybir.dt
    AF = mybir.ActivationFunctionType
    ALU = mybir.AluOpType

    nc = bass.Bass()
    # Batch-0-pivot softmax: g_b = h_b - h_0 (b=1..3) computed by ONE
    # full-K=128 matmul each: lhsT = [K_b^T ; K_0^T], rhs = [Q_b^T ; -Q_0^T]
    # (host packs/negates). Then w_b = e^{g_b/4}/(1 + sum e^{g_b'/4}) and
    # w_0 = 1/(1 + sum ...) = r, so batch 0 needs no exp and no multiply.
    qt = nc.declare_dram_parameter("qt", [3, HL, 128, S], DT.bfloat16, isOutput=False)
    kt = nc.declare_dram_parameter("kt", [3, HL, 128, S], DT.bfloat16, isOutput=False)
    vv = nc.declare_dram_parameter("v", [B, HL, S, D], DT.bfloat16, isOutput=False)
    # negated V: on DVE-reciprocal iterations the Newton chain produces -r
    # (one op shorter); pairing those weights with -V cancels the sign in
    # the PV accumulation.
    vn = nc.declare_dram_parameter("vn", [B, HL, S, D], DT.bfloat16, isOutput=False)
    out = nc.declare_dram_parameter("out", [B, HL, D, S], DT.float32, isOutput=True)

    with tile.TileContext(nc) as tc:
        with (
            tc.tile_pool(name="inputs", bufs=1) as ipool,
            tc.tile_pool(name="work", bufs=8) as wpool,
            tc.tile_pool(name="outsb", bufs=4) as opool,
            tc.tile_pool(name="qkps", bufs=2, space="PSUM") as qkpool,
            tc.tile_pool(name="ops", bufs=1, space="PSUM") as opsum,
        ):
            QT = ipool.tile([128, 3 * HL * S], DT.bfloat16, tag="qt")
            KT = ipool.tile([128, 3 * HL * S], DT.bfloat16, tag="kt")
            VA = ipool.tile([128, B * HL * NB * D], DT.bfloat16, tag="va")
            VN = ipool.tile([128, B * HL * NB * D], DT.bfloat16, tag="vn")
            for bb in range(3):
                for hl in range(HL):
                    off = (bb * HL + hl) * S
                    nc.sync.dma_start(out=QT[:, off : off + S], in_=qt[bb, hl])
                    nc.sync.dma_start(out=KT[:, off : off + S], in_=kt[bb, hl])
            for b in range(B):
                for hl in range(HL):
                    voff = (b * HL + hl) * NB * D
                    nc.sync.dma_start(
                        out=VA[:, voff : voff + NB * D].rearrange(
                            "p (n d) -> p n d", d=D
                        ),
                        in_=vv[b, hl].rearrange("(n p) d -> p n d", p=128),
                    )
                    nc.sync.dma_start(
                        out=VN[:, voff : voff + NB * D].rearrange(
                            "p (n d) -> p n d", d=D
                        ),
                        in_=vn[b, hl].rearrange("(n p) d -> p n d", p=128),
                    )

            for hl in range(HL):
                for ic in range(NI):
                    po = [
                        opsum.tile(
                            [128, 512], DT.float32, tag=f"po{p}", name=f"po{p}"
                        )
                        for p in range(2)
                    ]
                    for jb in range(NB):
                        # [128, 3*512] = 3 banks holding g_1|g_2|g_3;
                        # bufs=2 double-buffers the QK->exp handoff
                        qk = qkpool.tile([128, 1536], DT.float32, tag="qk")
                        for bb in range(3):
                            off = (bb * HL + hl) * S
                            nc.tensor.matmul(
                                qk[:, bb * 512 : (bb + 1) * 512],
                                lhsT=KT[:, off + jb * 128 : off + jb * 128 + 128],
                                rhs=QT[:, off + ic * 512 : off + ic * 512 + 512],
                                start=True,
                                stop=True,
                            )
                        E = wpool.tile([128, 1536], DT.bfloat16, tag="E")
                        nc.scalar.activation(E, qk, AF.Exp, scale=0.25)
                        T1 = wpool.tile([128, 512], DT.bfloat16, tag="T1")
                        nc.vector.tensor_add(T1, E[:, 0:512], E[:, 512:1024])
                        den = wpool.tile([128, 512], DT.bfloat16, tag="den")
                        # den = (E3 + 1.0) + T1 in one fused op
                        nc.vector.scalar_tensor_tensor(
                            out=den,
                            in0=E[:, 1024:1536],
                            scalar=1.0,
                            in1=T1,
                            op0=ALU.add,
                            op1=ALU.add,
                        )
                        rb = wpool.tile([128, 512], DT.bfloat16, tag="rb")
                        act_recip = jb % 2 == 0
                        if act_recip:
                            # reciprocal on ACT: r = exp(-ln(den)); Ln and Exp
                            # share the natural_log_exp table set
                            lnt = wpool.tile([128, 512], DT.float32, tag="lnt")
                            nc.scalar.activation(lnt, den, AF.Ln)
                            nc.scalar.activation(rb, lnt, AF.Exp, scale=-1.0)
                        else:
                            # reciprocal on DVE: bf16 exponent-flip seed + one
                            # Newton iteration, fused to produce -r (the PV
                            # matmuls below compensate with lhsT = -V)
                            ti = wpool.tile([128, 512], DT.bfloat16, tag="ti")
                            nc.vector.tensor_scalar(
                                out=ti.bitcast(DT.int16),
                                in0=den.bitcast(DT.int16),
                                scalar1=-1,
                                scalar2=None,
                                op0=ALU.bitwise_xor,
                            )
                            x0 = wpool.tile([128, 512], DT.bfloat16, tag="x0")
                            nc.vector.tensor_scalar(
                                out=x0.bitcast(DT.int16),
                                in0=ti.bitcast(DT.int16),
                                scalar1=0x7EF4,
                                scalar2=None,
                                op0=ALU.add,
                            )
                            dx = wpool.tile([128, 512], DT.bfloat16, tag="dx")
                            nc.vector.tensor_mul(dx, den, x0)
                            # rb = (dx - 2) * x0 = -x0*(2 - dx) = -r
                            nc.vector.scalar_tensor_tensor(
                                out=rb,
                                in0=dx,
                                scalar=2.0,
                                in1=x0,
                                op0=ALU.subtract,
                                op1=ALU.mult,
                            )
                        W = wpool.tile([128, 1536], DT.bfloat16, tag="W")
                        nc.vector.tensor_mul(
                            W.rearrange("q (b n) -> q b n", b=3),
                            E.rearrange("q (b n) -> q b n", b=3),
                            rb.unsqueeze(1).broadcast_to([128, 3, 512]),
                        )
                        # PV rhs per batch: b0 -> rb (w_0 = r), b1..b3 -> W
                        rhss = [rb, W[:, 0:512], W[:, 512:1024], W[:, 1024:1536]]
                        VV = VA if act_recip else VN
                        for p in range(2):
                            for half in range(2):
                                b = 2 * p + half
                                voff = (b * HL + hl) * NB * D + jb * D
                                nc.tensor.matmul(
                                    po[p][64 * half : 64 * (half + 1), :],
                                    lhsT=VV[:, voff : voff + D],
                                    rhs=rhss[b],
                                    start=(jb == 0),
                                    stop=(jb == NB - 1),
                                    tile_position=(0, 64 * half),
                                )
                    for p in range(2):
                        for half in range(2):
                            b = 2 * p + half
                            osb = opool.tile([D, 512], DT.float32, tag="osb")
                            # ACT, not DVE: DVE is the bottleneck engine and
                            # ScalarE reads PSUM faster anyway
                            nc.scalar.copy(
                                osb, po[p][64 * half : 64 * (half + 1), :]
                            )
                            nc.sync.dma_start(
                                out=out[b, hl, :, ic * 512 : (ic + 1) * 512],
                                in_=osb,
                            )
    return nc


def _patch_bir_waits(bir_json: bytes) -> bytes:
    """This walrus build only accepts 1 sync wait per instruction (2 for
    DMACopy); Tile emits more. Legalize:
      1. merge duplicate-semaphore waits (keep max threshold),
      2. drop waits that are transitively implied (vector-clock replay over
         the straight-line program: in-order completion per engine, FIFO per
         DMA queue, and the knowledge a producer had when it bumped a sem),
      3. split any residual multi-wait onto injected EventSemaphore
         instructions on the same engine right before the instruction.
    Only monotonic sem-inc/sem-ge-imm semaphores participate in (2); barrier
    sems (dec/eq) are left untouched."""
    import json
    from collections import defaultdict

    bir = json.loads(bir_json)

    for fn in bir["functions"]:
        insts = []
        for bb in fn["blocks"]:
            for inst in bb.get("instructions", []):
                insts.append(inst)

        # classify sems: monotonic = all updates are positive sem-inc and
        # all waits are sem-ge-imm
        bad_sems = set()
        for inst in insts:
            si = inst.get("sync_info") or {}
            for u in si.get("on_update") or []:
                if u.get("update_mode") != "sem-inc" or u.get("update_value", 0) <= 0:
                    bad_sems.add(u["id"])
            for w in si.get("on_wait") or []:
                if w.get("wait_mode") != "sem-ge-imm":
                    bad_sems.add(w["id"])

        # proc of an instruction: its engine stream, except DMACopy whose
        # completion (and sem update) is FIFO per DMA queue, keyed by the
        # sem it updates.
        def proc_of(inst):
            if inst.get("opcode") == "DMACopy":
                si = inst.get("sync_info") or {}
                ups = si.get("on_update") or []
                if ups:
                    return ("dma", ups[0]["id"])
            return ("eng", inst.get("engine"))

        sem_val = defaultdict(int)          # current cumulative value per sem
        producers = defaultdict(list)       # sem -> [(value_after, CK dict)]
        know = defaultdict(dict)            # proc -> {sem: guaranteed min}

        def join(dst, src):
            for s, v in src.items():
                if dst.get(s, 0) < v:
                    dst[s] = v

        out_blocks = {id(bb): [] for bb in fn["blocks"]}
        inj = 0
        for bb in fn["blocks"]:
            new_list = []
            for inst in bb.get("instructions", []):
                p = proc_of(inst)
                eng_p = ("eng", inst.get("engine"))
                # waits on a DMACopy are enforced by the DGE queue (FIFO per
                # queue), not the issuing engine — track knowledge per queue
                kp = p if p[0] == "dma" else eng_p
                si = inst.get("sync_info") or {}
                waits = si.get("on_wait") or []
                # merge duplicate sems
                merged = {}
                for w in waits:
                    k = w["id"]
                    if k not in merged or w.get("wait_value", 0) > merged[k].get(
                        "wait_value", 0
                    ):
                        merged[k] = w
                waits = list(merged.values())
                kept = []
                for w in waits:
                    s, v = w["id"], w.get("wait_value", 0)
                    if s in bad_sems:
                        kept.append(w)
                        continue
                    if know[kp].get(s, 0) >= v:
                        continue  # redundant
                    kept.append(w)
                    know[kp][s] = max(know[kp].get(s, 0), v)
                    # transitive knowledge from the producer that reached v
                    for val_after, ck in producers[s]:
                        if val_after >= v:
                            join(know[kp], ck)
                            break
                # split if too many waits remain
                budget = 1
                while len(kept) > budget:
                    w = kept.pop(0)
                    inj += 1
                    new_list.append(
                        {
                            "debug": inst.get("debug", 0),
                            "engine": inst.get("engine"),
                            "ins": [],
                            "name": f"WS-{inj}-{inst.get('name')}",
                            "opcode": "EventSemaphore",
                            "outs": [],
                            "sync_info": {"on_update": [], "on_wait": [w]},
                        }
                    )
                si["on_wait"] = kept
                inst["sync_info"] = si
                new_list.append(inst)
                # apply this instruction's updates for downstream knowledge
                ups = si.get("on_update") or []
                ck = None
                for u in ups:
                    s = u["id"]
                    if s in bad_sems:
                        continue
                    sem_val[s] += u.get("update_value", 0)
                    if ck is None:
                        # completion knowledge: what this proc knew here
                        # (for DMA: queue knowledge + engine state at issue)
                        ck = dict(know[kp])
                        if p[0] == "dma":
                            join(ck, know[eng_p])
                    ck[s] = sem_val[s]
                    producers[s].append((sem_val[s], ck))
                # a proc knows its own sems' values after completion
                if p[0] == "eng":
                    for u in ups:
                        if u["id"] not in bad_sems:
                            know[eng_p][u["id"]] = sem_val[u["id"]]
            out_blocks[id(bb)] = new_list
        for bb in fn["blocks"]:
            bb["instructions"] = out_blocks[id(bb)]
    return json.dumps(bir).encode()


_PATCHED = False


def _install_bir_patch():
    global _PATCHED
    if _PATCHED:
        return
    import concourse.bass2jax as bass2jax
    from concourse import bass_utils as _bu

    orig = _bu.compile_bir_kernel

    def patched(bir_json, tmpdir, neff_name="file.neff"):
        return orig(_patch_bir_waits(bir_json), tmpdir, neff_name)

    bass2jax.compile_bir_kernel = patched
    # keep profile artifacts local — no bucket in this environment
    _bu.upload_artifacts = lambda tmpdir: str(tmpdir)
    _PATCHED = True


def _install_ntff_shim():
    """run_bass_kernel_spmd(trace=True) under axon needs
    antenv.axon_hooks.get_axon_ntff_profile_hook; the module isn't staged in
    this image, but libaxon_pjrt.so exposes the profile C ABI — recreate the
    shim (same recipe as trn_agent_boot)."""
    import sys as _sys

    if "antenv.axon_hooks" in _sys.modules:
        return
    import contextlib
    import ctypes
    import types

    import antenv  # noqa: F401

    so_path = "/opt/axon/libaxon_pjrt.so"
    hook = None
    try:
        lib = ctypes.CDLL(so_path)
        if hasattr(lib, "axon_start_nrt_profile"):
            lib.axon_start_nrt_profile.argtypes = [
                ctypes.POINTER(ctypes.c_int64),
                ctypes.c_size_t,
            ]
            lib.axon_start_nrt_profile.restype = ctypes.c_int64
            lib.axon_stop_nrt_profile.argtypes = [ctypes.c_char_p]
            lib.axon_stop_nrt_profile.restype = ctypes.c_int64

            @contextlib.contextmanager
            def hook(output_dir, device_ids):
                import jax

                jax.devices()
                if device_ids:
                    ids = (ctypes.c_int64 * len(device_ids))(*device_ids)
                    rc = lib.axon_start_nrt_profile(ids, len(device_ids))
                else:
                    rc = lib.axon_start_nrt_profile(None, 0)
                if rc != 0:
                    raise RuntimeError(f"axon_start_nrt_profile rc={rc}")
                try:
                    yield
                finally:
                    n = lib.axon_stop_nrt_profile(str(output_dir).encode())
                    print(
                        f"ntff profile: {n} file(s) -> {output_dir}",
                        file=_sys.stderr,
                    )
    except OSError:
        pass

    mod = types.ModuleType("antenv.axon_hooks")
    mod.get_axon_ntff_profile_hook = lambda: hook
    mod.set_axon_ntff_profile_hook = lambda h: None
    _sys.modules["antenv.axon_hooks"] = mod
    import antenv as _ae

    _ae.axon_hooks = mod


def kernel(query, key, value, mask=None):
    global _NC, LAST_EXEC_NS, LAST_RESULTS
    from concourse.bass_utils import run_bass_kernel_spmd

    _install_bir_patch()
    if TRACE:
        _install_ntff_shim()

    query = np.asarray(query, dtype=np.float32)
    key = np.asarray(key, dtype=np.float32)
    value = np.asarray(value, dtype=np.float32)

    if _NC is None:
        _NC = _build_nc()
    nc = _NC

    bf16 = ml_dtypes.bfloat16

    def pack_pivot(x, negate_base):
        # [B, HL, S, D] -> [B, HL, D, S]; stack [x_b^T ; (+-)x_0^T] on the
        # partition axis for b = 1..3 -> [3, HL, 128, S]
        xt = x.transpose(0, 1, 3, 2)  # [B, HL, D, S]
        base = -xt[0] if negate_base else xt[0]  # [HL, D, S]
        stk = np.stack(
            [np.concatenate([xt[b], base], axis=1) for b in (1, 2, 3)], axis=0
        )
        return np.ascontiguousarray(stk).astype(bf16)

    in_maps = []
    for c in range(NCORES):
        hs = slice(HL * c, HL * (c + 1))
        qt = pack_pivot(query[:, hs], negate_base=True)
        kt = pack_pivot(key[:, hs], negate_base=False)
        vc = np.ascontiguousarray(value[:, hs]).astype(bf16)
        vnc = np.ascontiguousarray(-value[:, hs]).astype(bf16)
        in_maps.append({"qt": qt, "kt": kt, "v": vc, "vn": vnc})

    res = run_bass_kernel_spmd(
        nc, in_maps, core_ids=list(range(NCORES)), trace=TRACE
    )
    LAST_RESULTS = res
    LAST_EXEC_NS = getattr(res, "exec_time_ns", None)

    full = np.empty((B, H, S, D), dtype=np.float32)
    for c in range(NCORES):
        o = np.asarray(res.results[c]["out"])  # [B, HL, D, S]
        full[:, HL * c : HL * (c + 1)] = o.transpose(0, 1, 3, 2)
    return full


# revision 37
# speedup vs baseline: 1.2334x; 1.1930x over previous
"""Trainium2 Bass kernel for nn_Attention_10711648436709.

Math (faithful to reference):
    h = einsum('bhik,bhjk->bhij', Q, K) / sqrt(H)     # scale = sqrt(16) = 4
    w = softmax(h, axis=0)                            # over the BATCH axis (B=4)
    out = einsum('bhij,bhjv->bhiv', w, V)
    (mask is a no-op in the reference)

Sharding: head-parallel across 8 cores (16 heads -> 2 heads/core).
Softmax over batch stays core-local => communication-free.

Per-core layout trick: compute transposed scores S^T[j, i] so that
 - QK:  lhsT = K^T[d, j-block]  rhs = Q^T[d, i-chunk]   (host pre-transposes Q,K)
 - PV:  lhsT = V[j-block, v]    rhs = W[j, i-chunk]     (V in natural layout)
 - output accumulates as out^T[v, i] in PSUM; host transposes back.
Softmax over batch is elementwise across the 4 per-batch score planes that
live side-by-side in one [128, 4*512] PSUM mega-tile (one exp covers all 4).
"""

import sys
import os

for p in ("/opt/trn_rl_repo",):
    if p not in sys.path:
        sys.path.insert(0, p)

import numpy as np
import ml_dtypes

B, H, S, D = 4, 16, 2048, 64
NCORES = 8
HL = H // NCORES          # 2 heads per core
NB = S // 128             # 16 j-blocks
NI = S // 512             # 4 i-chunks

TRACE = False
LAST_EXEC_NS = None
LAST_RESULTS = None

_NC = None


def _build_nc():
    import concourse.bass as bass
    import concourse.mybir as mybir
    import concourse.tile as tile

    DT = mybir.dt
    AF = mybir.ActivationFunctionType
    ALU = mybir.AluOpType

    nc = bass.Bass()
    # Batch-0-pivot softmax: g_b = h_b - h_0 (b=1..3) computed by ONE
    # full-K=128 matmul each: lhsT = [K_b^T ; K_0^T], rhs = [Q_b^T ; -Q_0^T]
    # (host packs/negates). Then w_b = e^{g_b/4}/(1 + sum e^{g_b'/4}) and
    # w_0 = 1/(1 + sum ...) = r, so batch 0 needs no exp and no multiply.
    qt = nc.declare_dram_parameter("qt", [3, HL, 128, S], DT.bfloat16, isOutput=False)
    kt = nc.declare_dram_parameter("kt", [3, HL, 128, S], DT.bfloat16, isOutput=False)
    vv = nc.declare_dram_parameter("v", [B, HL, S, D], DT.bfloat16, isOutput=False)
    # negated V: on DVE-reciprocal iterations the Newton chain produces -r
    # (one op shorter); pairing those weights with -V cancels the sign in
    # the PV accumulation.
    vn = nc.declare_dram_parameter("vn", [B, HL, S, D], DT.bfloat16, isOutput=False)
    out = nc.declare_dram_parameter("out", [B, HL, D, S], DT.float32, isOutput=True)

    with tile.TileContext(nc) as tc:
        with (
            tc.tile_pool(name="inputs", bufs=1) as ipool,
            tc.tile_pool(name="work", bufs=6) as wpool,
            tc.tile_pool(name="outsb", bufs=6) as opool,
            tc.tile_pool(name="qkps", bufs=2, space="PSUM") as qkpool,
            tc.tile_pool(name="ops", bufs=1, space="PSUM") as opsum,
        ):
            QT = ipool.tile([128, 3 * HL * S], DT.bfloat16, tag="qt")
            KT = ipool.tile([128, 3 * HL * S], DT.bfloat16, tag="kt")
            VA = ipool.tile([128, B * HL * NB * D], DT.bfloat16, tag="va")
            VN = ipool.tile([128, B * HL * NB * D], DT.bfloat16, tag="vn")
            for bb in range(3):
                for hl in range(HL):
                    off = (bb * HL + hl) * S
                    nc.sync.dma_start(out=QT[:, off : off + S], in_=qt[bb, hl])
                    nc.sync.dma_start(out=KT[:, off : off + S], in_=kt[bb, hl])
            for b in range(B):
                for hl in range(HL):
                    voff = (b * HL + hl) * NB * D
                    nc.sync.dma_start(
                        out=VA[:, voff : voff + NB * D].rearrange(
                            "p (n d) -> p n d", d=D
                        ),
                        in_=vv[b, hl].rearrange("(n p) d -> p n d", p=128),
                    )
                    nc.sync.dma_start(
                        out=VN[:, voff : voff + NB * D].rearrange(
                            "p (n d) -> p n d", d=D
                        ),
                        in_=vn[b, hl].rearrange("(n p) d -> p n d", p=128),
                    )

            for hl in range(HL):
                for ic in range(NI):
                    po = [
                        opsum.tile(
                            [128, 512], DT.float32, tag=f"po{p}", name=f"po{p}"
                        )
                        for p in range(2)
                    ]
                    for jb in range(NB):
                        # [128, 3*512] = 3 banks holding g_1|g_2|g_3;
                        # bufs=2 double-buffers the QK->exp handoff
                        qk = qkpool.tile([128, 1536], DT.float32, tag="qk")
                        for bb in range(3):
                            off = (bb * HL + hl) * S
                            nc.tensor.matmul(
                                qk[:, bb * 512 : (bb + 1) * 512],
                                lhsT=KT[:, off + jb * 128 : off + jb * 128 + 128],
                                rhs=QT[:, off + ic * 512 : off + ic * 512 + 512],
                                start=True,
                                stop=True,
                            )
                        E = wpool.tile([128, 1536], DT.bfloat16, tag="E")
                        nc.scalar.activation(E, qk, AF.Exp, scale=0.25)
                        T1 = wpool.tile([128, 512], DT.bfloat16, tag="T1")
                        nc.vector.tensor_add(T1, E[:, 0:512], E[:, 512:1024])
                        den = wpool.tile([128, 512], DT.bfloat16, tag="den")
                        # den = (E3 + 1.0) + T1 in one fused op
                        nc.vector.scalar_tensor_tensor(
                            out=den,
                            in0=E[:, 1024:1536],
                            scalar=1.0,
                            in1=T1,
                            op0=ALU.add,
                            op1=ALU.add,
                        )
                        rb = wpool.tile([128, 512], DT.bfloat16, tag="rb")
                        act_recip = jb % 2 == 0
                        if act_recip:
                            # reciprocal on ACT: r = exp(-ln(den)); Ln and Exp
                            # share the natural_log_exp table set
                            lnt = wpool.tile([128, 512], DT.float32, tag="lnt")
                            nc.scalar.activation(lnt, den, AF.Ln)
                            nc.scalar.activation(rb, lnt, AF.Exp, scale=-1.0)
                        else:
                            # reciprocal on DVE: bf16 exponent-flip seed + one
                            # Newton iteration, fused to produce -r (the PV
                            # matmuls below compensate with lhsT = -V)
                            ti = wpool.tile([128, 512], DT.bfloat16, tag="ti")
                            nc.vector.tensor_scalar(
                                out=ti.bitcast(DT.int16),
                                in0=den.bitcast(DT.int16),
                                scalar1=-1,
                                scalar2=None,
                                op0=ALU.bitwise_xor,
                            )
                            x0 = wpool.tile([128, 512], DT.bfloat16, tag="x0")
                            nc.vector.tensor_scalar(
                                out=x0.bitcast(DT.int16),
                                in0=ti.bitcast(DT.int16),
                                scalar1=0x7EF4,
                                scalar2=None,
                                op0=ALU.add,
                            )
                            dx = wpool.tile([128, 512], DT.bfloat16, tag="dx")
                            nc.vector.tensor_mul(dx, den, x0)
                            # rb = (dx - 2) * x0 = -x0*(2 - dx) = -r
                            nc.vector.scalar_tensor_tensor(
                                out=rb,
                                in0=dx,
                                scalar=2.0,
                                in1=x0,
                                op0=ALU.subtract,
                                op1=ALU.mult,
                            )
                        W = wpool.tile([128, 1536], DT.bfloat16, tag="W")
                        nc.vector.tensor_mul(
                            W.rearrange("q (b n) -> q b n", b=3),
                            E.rearrange("q (b n) -> q b n", b=3),
                            rb.unsqueeze(1).broadcast_to([128, 3, 512]),
                        )
                        # PV rhs per batch: b0 -> rb (w_0 = r), b1..b3 -> W
                        rhss = [rb, W[:, 0:512], W[:, 512:1024], W[:, 1024:1536]]
                        VV = VA if act_recip else VN
                        for p in range(2):
                            for half in range(2):
                                b = 2 * p + half
                                voff = (b * HL + hl) * NB * D + jb * D
                                nc.tensor.matmul(
                                    po[p][64 * half : 64 * (half + 1), :],
                                    lhsT=VV[:, voff : voff + D],
                                    rhs=rhss[b],
                                    start=(jb == 0),
                                    stop=(jb == NB - 1),
                                    tile_position=(0, 64 * half),
                                )
                    for p in range(2):
                        for half in range(2):
                            b = 2 * p + half
                            osb = opool.tile([D, 512], DT.float32, tag="osb")
                            # ACT, not DVE: DVE is the bottleneck engine and
                            # ScalarE reads PSUM faster anyway
                            nc.scalar.copy(
                                osb, po[p][64 * half : 64 * (half + 1), :]
                            )
                            nc.sync.dma_start(
                                out=out[b, hl, :, ic * 512 : (ic + 1) * 512],
                                in_=osb,
                            )
    return nc


def _patch_bir_waits(bir_json: bytes) -> bytes:
    """This walrus build only accepts 1 sync wait per instruction (2 for
    DMACopy); Tile emits more. Legalize:
      1. merge duplicate-semaphore waits (keep max threshold),
      2. drop waits that are transitively implied (vector-clock replay over
         the straight-line program: in-order completion per engine, FIFO per
         DMA queue, and the knowledge a producer had when it bumped a sem),
      3. split any residual multi-wait onto injected EventSemaphore
         instructions on the same engine right before the instruction.
    Only monotonic sem-inc/sem-ge-imm semaphores participate in (2); barrier
    sems (dec/eq) are left untouched."""
    import json
    from collections import defaultdict

    bir = json.loads(bir_json)

    for fn in bir["functions"]:
        insts = []
        for bb in fn["blocks"]:
            for inst in bb.get("instructions", []):
                insts.append(inst)

        # classify sems: monotonic = all updates are positive sem-inc and
        # all waits are sem-ge-imm
        bad_sems = set()
        for inst in insts:
            si = inst.get("sync_info") or {}
            for u in si.get("on_update") or []:
                if u.get("update_mode") != "sem-inc" or u.get("update_value", 0) <= 0:
                    bad_sems.add(u["id"])
            for w in si.get("on_wait") or []:
                if w.get("wait_mode") != "sem-ge-imm":
                    bad_sems.add(w["id"])

        # proc of an instruction: its engine stream, except DMACopy whose
        # completion (and sem update) is FIFO per DMA queue, keyed by the
        # sem it updates.
        def proc_of(inst):
            if inst.get("opcode") == "DMACopy":
                si = inst.get("sync_info") or {}
                ups = si.get("on_update") or []
                if ups:
                    return ("dma", ups[0]["id"])
            return ("eng", inst.get("engine"))

        sem_val = defaultdict(int)          # current cumulative value per sem
        producers = defaultdict(list)       # sem -> [(value_after, CK dict)]
        know = defaultdict(dict)            # proc -> {sem: guaranteed min}

        def join(dst, src):
            for s, v in src.items():
                if dst.get(s, 0) < v:
                    dst[s] = v

        out_blocks = {id(bb): [] for bb in fn["blocks"]}
        inj = 0
        for bb in fn["blocks"]:
            new_list = []
            for inst in bb.get("instructions", []):
                p = proc_of(inst)
                eng_p = ("eng", inst.get("engine"))
                # waits on a DMACopy are enforced by the DGE queue (FIFO per
                # queue), not the issuing engine — track knowledge per queue
                kp = p if p[0] == "dma" else eng_p
                si = inst.get("sync_info") or {}
                waits = si.get("on_wait") or []
                # merge duplicate sems
                merged = {}
                for w in waits:
                    k = w["id"]
                    if k not in merged or w.get("wait_value", 0) > merged[k].get(
                        "wait_value", 0
                    ):
                        merged[k] = w
                waits = list(merged.values())
                kept = []
                for w in waits:
                    s, v = w["id"], w.get("wait_value", 0)
                    if s in bad_sems:
                        kept.append(w)
                        continue
                    if know[kp].get(s, 0) >= v:
                        continue  # redundant
                    kept.append(w)
                    know[kp][s] = max(know[kp].get(s, 0), v)
                    # transitive knowledge from the producer that reached v
                    for val_after, ck in producers[s]:
                        if val_after >= v:
                            join(know[kp], ck)
                            break
                # split if too many waits remain
                budget = 1
                while len(kept) > budget:
                    w = kept.pop(0)
                    inj += 1
                    new_list.append(
                        {
                            "debug": inst.get("debug", 0),
                            "engine": inst.get("engine"),
                            "ins": [],
                            "name": f"WS-{inj}-{inst.get('name')}",
                            "opcode": "EventSemaphore",
                            "outs": [],
                            "sync_info": {"on_update": [], "on_wait": [w]},
                        }
                    )
                si["on_wait"] = kept
                inst["sync_info"] = si
                new_list.append(inst)
                # apply this instruction's updates for downstream knowledge
                ups = si.get("on_update") or []
                ck = None
                for u in ups:
                    s = u["id"]
                    if s in bad_sems:
                        continue
                    sem_val[s] += u.get("update_value", 0)
                    if ck is None:
                        # completion knowledge: what this proc knew here
                        # (for DMA: queue knowledge + engine state at issue)
                        ck = dict(know[kp])
                        if p[0] == "dma":
                            join(ck, know[eng_p])
                    ck[s] = sem_val[s]
                    producers[s].append((sem_val[s], ck))
                # a proc knows its own sems' values after completion
                if p[0] == "eng":
                    for u in ups:
                        if u["id"] not in bad_sems:
                            know[eng_p][u["id"]] = sem_val[u["id"]]
            out_blocks[id(bb)] = new_list
        for bb in fn["blocks"]:
            bb["instructions"] = out_blocks[id(bb)]
    return json.dumps(bir).encode()


_PATCHED = False


def _install_bir_patch():
    global _PATCHED
    if _PATCHED:
        return
    import concourse.bass2jax as bass2jax
    from concourse import bass_utils as _bu

    orig = _bu.compile_bir_kernel

    def patched(bir_json, tmpdir, neff_name="file.neff"):
        return orig(_patch_bir_waits(bir_json), tmpdir, neff_name)

    bass2jax.compile_bir_kernel = patched
    # keep profile artifacts local — no bucket in this environment
    _bu.upload_artifacts = lambda tmpdir: str(tmpdir)
    _PATCHED = True


def _install_ntff_shim():
    """run_bass_kernel_spmd(trace=True) under axon needs
    antenv.axon_hooks.get_axon_ntff_profile_hook; the module isn't staged in
    this image, but libaxon_pjrt.so exposes the profile C ABI — recreate the
    shim (same recipe as trn_agent_boot)."""
    import sys as _sys

    if "antenv.axon_hooks" in _sys.modules:
        return
    import contextlib
    import ctypes
    import types

    import antenv  # noqa: F401

    so_path = "/opt/axon/libaxon_pjrt.so"
    hook = None
    try:
        lib = ctypes.CDLL(so_path)
        if hasattr(lib, "axon_start_nrt_profile"):
            lib.axon_start_nrt_profile.argtypes = [
                ctypes.POINTER(ctypes.c_int64),
                ctypes.c_size_t,
            ]
            lib.axon_start_nrt_profile.restype = ctypes.c_int64
            lib.axon_stop_nrt_profile.argtypes = [ctypes.c_char_p]
            lib.axon_stop_nrt_profile.restype = ctypes.c_int64

            @contextlib.contextmanager
            def hook(output_dir, device_ids):
                import jax

                jax.devices()
                if device_ids:
                    ids = (ctypes.c_int64 * len(device_ids))(*device_ids)
                    rc = lib.axon_start_nrt_profile(ids, len(device_ids))
                else:
                    rc = lib.axon_start_nrt_profile(None, 0)
                if rc != 0:
                    raise RuntimeError(f"axon_start_nrt_profile rc={rc}")
                try:
                    yield
                finally:
                    n = lib.axon_stop_nrt_profile(str(output_dir).encode())
                    print(
                        f"ntff profile: {n} file(s) -> {output_dir}",
                        file=_sys.stderr,
                    )
    except OSError:
        pass

    mod = types.ModuleType("antenv.axon_hooks")
    mod.get_axon_ntff_profile_hook = lambda: hook
    mod.set_axon_ntff_profile_hook = lambda h: None
    _sys.modules["antenv.axon_hooks"] = mod
    import antenv as _ae

    _ae.axon_hooks = mod


def kernel(query, key, value, mask=None):
    global _NC, LAST_EXEC_NS, LAST_RESULTS
    from concourse.bass_utils import run_bass_kernel_spmd

    _install_bir_patch()
    if TRACE:
        _install_ntff_shim()

    query = np.asarray(query, dtype=np.float32)
    key = np.asarray(key, dtype=np.float32)
    value = np.asarray(value, dtype=np.float32)

    if _NC is None:
        _NC = _build_nc()
    nc = _NC

    bf16 = ml_dtypes.bfloat16

    def pack_pivot(x, negate_base):
        # [B, HL, S, D] -> [B, HL, D, S]; stack [x_b^T ; (+-)x_0^T] on the
        # partition axis for b = 1..3 -> [3, HL, 128, S]
        xt = x.transpose(0, 1, 3, 2)  # [B, HL, D, S]
        base = -xt[0] if negate_base else xt[0]  # [HL, D, S]
        stk = np.stack(
            [np.concatenate([xt[b], base], axis=1) for b in (1, 2, 3)], axis=0
        )
        return np.ascontiguousarray(stk).astype(bf16)

    in_maps = []
    for c in range(NCORES):
        hs = slice(HL * c, HL * (c + 1))
        qt = pack_pivot(query[:, hs], negate_base=True)
        kt = pack_pivot(key[:, hs], negate_base=False)
        vc = np.ascontiguousarray(value[:, hs]).astype(bf16)
        vnc = np.ascontiguousarray(-value[:, hs]).astype(bf16)
        in_maps.append({"qt": qt, "kt": kt, "v": vc, "vn": vnc})

    res = run_bass_kernel_spmd(
        nc, in_maps, core_ids=list(range(NCORES)), trace=TRACE
    )
    LAST_RESULTS = res
    LAST_EXEC_NS = getattr(res, "exec_time_ns", None)

    full = np.empty((B, H, S, D), dtype=np.float32)
    for c in range(NCORES):
        o = np.asarray(res.results[c]["out"])  # [B, HL, D, S]
        full[:, HL * c : HL * (c + 1)] = o.transpose(0, 1, 3, 2)
    return full
